# revision 42
# baseline (speedup 1.0000x reference)
"""Trainium2 Bass kernel for the dynamic-kernel ECA module.

Computation per sample:
  gap  = mean(x, axis=l)                       (c,)
  h    = gelu(gap @ w1.T + b1)                 (hidden,)
  th   = tanh(h @ w2.T + b2); delta = 2*th     scalar
  k    = (5 + clip(round(delta), -3, 3)) | 1   in {3,5,7} (delta in (-2,2))
  w    = box filter of width k in 9-tap window, 1/k weights
  y    = conv1d(gap, w) along c (zero pad 4)   (c,)
  s    = sigmoid(y)
  out  = x * s[:, None]

Sharding: pure data parallel, batch 16 -> 8 cores x 2 samples.

The kernel is HBM-bandwidth-bound: 16 MiB of bf16 x in + 16 MiB out per
core at the ~427 GB/s/core SDMA rate (16 engines x ~26.7 GB/s) is
~78.6 us of pure data movement, plus ~7.5 us fixed NEFF preamble.
Everything else hides under the DMA stream:

  * ALL x traffic (16 x 1 MiB loads, then 9 stores) rides the HWDGE
    rings with 15 loads + all stores FIFO on the sync ring: store
    descriptors are enqueued long before the loads drain, so the ring
    never idles across the load->store transition.  The first half-
    load plus the ~176 KiB fp8 constant blob ride the scalar ring so
    both rings feed the SDMA engines during spin-up.  The last store
    is split in two 1 MiB pieces so the final completion receipt
    trails less data.  (Keep transfers at [128, >=4096]: odd-shaped
    ones get dealt to the 16 SDMA engines very unevenly and trickle
    out on one engine.)
  * l-sum reductions are split per 1 MiB half-tile: one half goes to
    VectorE as a fused fold+reduce (scalar_tensor_tensor with
    accum_out: one pass over 2048 bf16 pairs, ~2.3 us), the other to
    ScalarE as an activation-accumulate (~3.7 us).  For the last chunk
    of each sample the assignment is swapped so the final (merge-
    gating) reduce is the cheap VectorE one.
  * The per-sample gate: MLP and the nine candidate band-conv matmuls
    on the PE (fp8 host-precomputed constant lhsT - raw w1 plus exact
    0/1 band masks, with 1/L and 1/(k*L) folded into the activation
    scales; mixed fp8-lhsT x bf16-rhs matmul is legal, while an
    on-chip DVE f32->fp8 cast of the gap NaNs on HW - avoid; a
    zero-padded [128, CP+2] gap layout turns chunk-boundary wraps
    into column-shifted rhs views), gelu/tanh and the candidate
    tk_k = tanh(y_k/2) on ScalarE, and a 3-way flag blend on VectorE
    (sigmoid(y) = 0.5 + 0.5*tanh(y/2), flags sum to 1).  Dummy
    Gelu/Tanh activations at program start hoist the ACT table loads
    off the gate critical path.  (Engine APs must start at partition
    0/32/64/96, so a VectorE partition-shift conv is not legal.)
  * VectorE's stream is deadline-ordered: sample-1 reduces are
    interleaved between sample-0 scale muls so every store's scale
    lands several us before the ring reaches that store.

Known environment hazard: SDMA engine 15 intermittently runs ~20%
slow (port contention outside the kernel's control).  Descriptors
deal round-robin across all 16 engines, so 1/16 of the traffic rides
it and a slow run costs ~17 us regardless of kernel structure; the
schedule above is byte-minimal to limit that exposure.
"""

import os
from contextlib import ExitStack

import numpy as np
import ml_dtypes

import concourse.bacc as bacc
import concourse.mybir as mybir
import concourse.tile as tile
from concourse.bass_utils import run_bass_kernel_spmd

F32 = mybir.dt.float32
BF16 = mybir.dt.bfloat16
FP8 = mybir.dt.float8e4     # ml_dtypes.float8_e4m3; 0/1 masks + w1 fit exactly
ALU = mybir.AluOpType
ACTF = mybir.ActivationFunctionType
AX_X = mybir.AxisListType.X

GELU = ACTF.Gelu             # simtest overrides (CoreSim lacks Gelu)

B, C, L = 16, 512, 8192
HID = 64
N_CORES = 8
BS = B // N_CORES            # samples per core = 2
CP = C // 128                # channel chunks = 4
HL = L // 2                  # 4096 (1 MiB bf16 half-tile)
QL = HL // 2                 # 2048

# packed bf16 const blob layout (columns)
W1T_OFF = 0                  # [128, CP*HID] = 256 cols
WB_OFF = 256                 # [128, 3*3*128] = 1152 cols (m-major, k-minor)
CB_COLS = 1408


def _build(b2_val):
    nc = bacc.Bacc("TRN2", target_bir_lowering=False, debug=False,
                   num_devices=N_CORES)

    x_d = nc.dram_tensor("x", [BS, C, L], BF16, kind="ExternalInput").ap()
    cstb_d = nc.dram_tensor("cstb", [128, CB_COLS], FP8,
                            kind="ExternalInput").ap()
    cstf_d = nc.dram_tensor("cstf", [HID, 2], F32, kind="ExternalInput").ap()
    o_d = nc.dram_tensor("out", [BS, C, L], BF16, kind="ExternalOutput").ap()

    with ExitStack() as ctx:
        tc = ctx.enter_context(tile.TileContext(nc))
        cache = ctx.enter_context(tc.tile_pool(name="cache", bufs=1))
        small = ctx.enter_context(tc.tile_pool(name="small", bufs=1))
        psum = ctx.enter_context(tc.tile_pool(name="psum", bufs=1, space="PSUM"))

        xt = {}

        # ---- init: memsets (V), ACT-table prefetch (A), const loads ------
        geP = {s: small.tile([128, CP + 2], BF16, tag=f"ge{s}", name=f"geP{s}")
               for s in range(BS)}
        partials = {s: small.tile([128, CP, 2], F32, tag=f"par{s}",
                                  name=f"par{s}") for s in range(BS)}
        ones = small.tile([1, 128], F32, tag="ones")
        b2t = small.tile([1, 1], F32, tag="b2t")
        dmy = small.tile([1, 3], F32, tag="dmy")
        for s in range(BS):
            nc.vector.memset(geP[s][:], 0.0)
        nc.vector.memset(ones[:], 1.0)
        nc.vector.memset(b2t[:], float(b2_val))
        nc.vector.memset(dmy[:], 0.0)
        # hoist ACT table loads off the gate critical path
        nc.scalar.activation(dmy[:, 1:2], dmy[:, 0:1], GELU)
        nc.scalar.activation(dmy[:, 2:3], dmy[:, 0:1], ACTF.Tanh)

        cstb = small.tile([128, CB_COLS], FP8, tag="cstb")
        cstf = small.tile([HID, 2], F32, tag="cstf")

        # ---- x-load triggers up front.  The very first half-load rides
        # ---- the scalar ring so both HWDGE rings feed the 16 SDMA
        # ---- engines during spin-up; the other 15 (and all stores)
        # ---- stay FIFO on the sync ring.  Consts follow on scalar. ------
        for s in range(BS):
            for ci in range(CP):
                t = cache.tile([128, L], BF16, tag=f"x{s}{ci}",
                               name=f"x{s}{ci}")
                xt[(s, ci)] = t
                for h in range(2):
                    eng = nc.scalar if (s, ci, h) == (0, 0, 0) else nc.sync
                    eng.dma_start(
                        out=t[:, h * HL:(h + 1) * HL],
                        in_=x_d[s, ci * 128:(ci + 1) * 128, h * HL:(h + 1) * HL])

        nc.scalar.dma_start(out=cstb[:], in_=cstb_d[:])
        nc.scalar.dma_start(out=cstf[:], in_=cstf_d[:])
        b1 = cstf[0:HID, 0:1]
        w2t = cstf[0:HID, 1:2]

        scratch = small.tile([128, QL], BF16, tag="stt_scratch")

        def red_v(s, ci, h):
            # fused fold+reduce of one half on VectorE
            t = xt[(s, ci)][:, h * HL:(h + 1) * HL]
            nc.vector.scalar_tensor_tensor(
                out=scratch[:], in0=t[:, 0:QL], scalar=1.0, in1=t[:, QL:HL],
                op0=ALU.mult, op1=ALU.add,
                accum_out=partials[s][:, ci, 0:1])

        def red_a(s, ci, h):
            t = xt[(s, ci)][:, h * HL:(h + 1) * HL]
            nc.scalar.activation(t, t, ACTF.Copy,
                                 accum_out=partials[s][:, ci, 1:2])

        geF = small.tile([128, CP], F32, tag="geF")

        def merge(s):
            # partials -> f32 merge, then bf16 cast into the padded layout
            nc.vector.reduce_sum(out=geF[:], in_=partials[s][:], axis=AX_X)
            nc.vector.tensor_scalar(out=geP[s][:, 1:1 + CP], in0=geF[:],
                                    scalar1=1.0, scalar2=None, op0=ALU.mult)

        # ---- gate pieces -------------------------------------------------
        def gate_mlp(s):
            # PE MLP + A activations; th broadcast across partitions
            hp = psum.tile([HID, 1], F32, tag="hp")
            for i in range(CP):
                nc.tensor.matmul(hp[:],
                                 lhsT=cstb[:, W1T_OFF + i * HID:
                                           W1T_OFF + (i + 1) * HID],
                                 rhs=geP[s][:, 1 + i:2 + i],
                                 start=(i == 0), stop=(i == CP - 1))
            h = small.tile([HID, 1], F32, tag="h")
            nc.scalar.activation(h[:], hp[:], GELU, bias=b1, scale=1.0 / L)
            dp = psum.tile([1, 1], F32, tag="dp")
            nc.tensor.matmul(dp[:], lhsT=h[:], rhs=w2t, start=True, stop=True)
            th = small.tile([1, 1], F32, tag="th")
            nc.scalar.activation(th[:], dp[:], ACTF.Tanh, bias=b2t[:],
                                 scale=1.0)
            thp = psum.tile([128, 1], F32, tag="thp")
            nc.tensor.matmul(thp[:], lhsT=ones[:], rhs=th[:], start=True,
                             stop=True)
            return thp

        def wb(m, kidx):
            o = WB_OFF + (m * 3 + kidx) * 128
            return cstb[:, o:o + 128]

        def gate_bands(s):
            # 9-tap band conv as constant-lhsT PE matmuls; the zero-padded
            # bf16 gap columns turn chunk-boundary wraps into column shifts
            yk = []
            for kidx in range(3):
                yp = psum.tile([128, CP], F32, tag=f"y{kidx}")
                nc.tensor.matmul(yp[:], lhsT=wb(0, kidx),
                                 rhs=geP[s][:, 1:1 + CP], start=True,
                                 stop=False)
                nc.tensor.matmul(yp[:], lhsT=wb(1, kidx),
                                 rhs=geP[s][:, 2:2 + CP], start=False,
                                 stop=False)
                nc.tensor.matmul(yp[:], lhsT=wb(2, kidx),
                                 rhs=geP[s][:, 0:CP], start=False, stop=True)
                yk.append(yp)
            return yk

        def gate_tks(yk):
            # tk_k = tanh(y_k / 2); wb holds exact fp8 0/1 masks so the
            # 1/(k*L) box weight lands in the tanh scale instead
            tk = []
            for yp, k in zip(yk, (3, 5, 7)):
                tt = small.tile([128, CP], F32, tag=f"tk{k}")
                nc.scalar.activation(tt[:], yp[:], ACTF.Tanh,
                                     scale=1.0 / (2.0 * k * L))
                tk.append(tt)
            return tk

        def gate_blend(s, tk, thp):
            # flags: a = [th >= 0.25] (k=7), bb = [th < -0.75] (k=3)
            fb = small.tile([128, 2], F32, tag="fb")
            nc.vector.tensor_scalar(out=fb[:, 0:1], in0=thp[:], scalar1=0.25,
                                    scalar2=None, op0=ALU.is_ge)
            nc.vector.tensor_scalar(out=fb[:, 1:2], in0=thp[:], scalar1=-0.75,
                                    scalar2=None, op0=ALU.is_lt)
            # bl = tk5 + a*(tk7-tk5) + bb*(tk3-tk5);  sg = 0.5 + 0.5*bl
            u = small.tile([128, CP], F32, tag="u")
            bl = small.tile([128, CP], F32, tag="bl")
            nc.vector.tensor_sub(u[:], tk[2][:], tk[1][:])
            nc.vector.scalar_tensor_tensor(out=bl[:], in0=u[:],
                                           scalar=fb[:, 0:1], in1=tk[1][:],
                                           op0=ALU.mult, op1=ALU.add)
            nc.vector.tensor_sub(u[:], tk[0][:], tk[1][:])
            nc.vector.scalar_tensor_tensor(out=bl[:], in0=u[:],
                                           scalar=fb[:, 1:2], in1=bl[:],
                                           op0=ALU.mult, op1=ALU.add)
            sg = small.tile([128, CP], F32, tag=f"sg{s}")
            nc.vector.tensor_scalar(out=sg[:], in0=bl[:], scalar1=0.5,
                                    scalar2=0.5, op0=ALU.mult, op1=ALU.add)
            return sg

        def scale_tile(s, ci, sg):
            t = xt[(s, ci)]
            nc.vector.tensor_scalar_mul(t[:], t[:], sg[:, ci:ci + 1])

        def store_tile(s, ci, split=False):
            t = xt[(s, ci)]
            cuts = [0, HL, L] if split else [0, L]
            for a, b in zip(cuts[:-1], cuts[1:]):
                nc.sync.dma_start(out=o_d[s, ci * 128:(ci + 1) * 128, a:b],
                                  in_=t[:, a:b])

        def red_sample(s):
            # last chunk swapped so the merge-gating reduce is VectorE's
            for ci in range(CP):
                if ci < CP - 1:
                    red_v(s, ci, 0)
                    red_a(s, ci, 1)
                else:
                    red_a(s, ci, 0)
                    red_v(s, ci, 1)

        # ---- sample 0: reduce + gate; then sample-0 scale/stores
        # ---- interleaved with sample-1 reduces (deadline order) ----------
        red_sample(0)
        merge(0)
        thp0 = gate_mlp(0)
        accs0 = gate_bands(0)
        tk0 = gate_tks(accs0)
        red_v(1, 0, 0)
        red_a(1, 0, 1)
        sg0 = gate_blend(0, tk0, thp0)

        scale_tile(0, 0, sg0)
        store_tile(0, 0)
        red_v(1, 1, 0)
        red_a(1, 1, 1)
        scale_tile(0, 1, sg0)
        store_tile(0, 1)
        red_v(1, 2, 0)
        red_a(1, 2, 1)
        scale_tile(0, 2, sg0)
        store_tile(0, 2)
        red_a(1, 3, 0)
        red_v(1, 3, 1)
        scale_tile(0, 3, sg0)
        store_tile(0, 3)

        merge(1)
        thp1 = gate_mlp(1)
        accs1 = gate_bands(1)
        tk1 = gate_tks(accs1)
        sg1 = gate_blend(1, tk1, thp1)
        for ci in range(CP):
            scale_tile(1, ci, sg1)
            store_tile(1, ci, split=(ci == CP - 1))

    nc.compile()
    return nc


_COMPILED = {}


def _get_compiled(b2_val):
    key = float(b2_val)
    if key not in _COMPILED:
        _COMPILED[key] = _build(key)
    return _COMPILED[key]


def _make_consts(w1, b1, w2, b2):
    w1 = np.asarray(w1, np.float32)
    b1 = np.asarray(b1, np.float32)
    w2 = np.asarray(w2, np.float32)

    S17 = np.zeros((128, 17, 128), np.float32)
    p = np.arange(128)
    for j in range(9):
        d = j - 4
        m = (p + d >= 0) & (p + d < 128)
        S17[p[m] + d, j, p[m]] = 1.0
    for d in range(1, 5):
        m = p + d - 128 >= 0
        S17[p[m] + d - 128, 8 + d, p[m]] = 1.0
    for d in range(-4, 0):
        m = p + d + 128 < 128
        S17[p[m] + d + 128, 17 + d, p[m]] = 1.0

    cstb = np.zeros((128, CB_COLS), np.float32)
    # w1t: [CP, 128, HID] flattened as CP blocks of HID columns.  The fp8
    # blob stores raw w1 (|w1| well inside fp8e4's +-240) and exact 0/1
    # band masks; 1/L and 1/(k*L) live in the activation scales.
    w1t = w1.T.reshape(CP, 128, HID)
    for i in range(CP):
        cstb[:, W1T_OFF + i * HID:W1T_OFF + (i + 1) * HID] = w1t[i]
    j9 = np.arange(9)
    for kidx, k in enumerate((3, 5, 7)):
        w = (np.abs(j9 - 4) <= (k - 1) // 2).astype(np.float32)
        bands = [sum(w[j] * S17[:, j, :] for j in range(9)),
                 sum(w[d + 4] * S17[:, 8 + d, :] for d in range(1, 5)),
                 sum(w[d + 4] * S17[:, 17 + d, :] for d in range(-4, 0))]
        for m in range(3):
            o = WB_OFF + (m * 3 + kidx) * 128
            cstb[:, o:o + 128] = bands[m]
    cstf = np.zeros((HID, 2), np.float32)
    cstf[:, 0] = b1
    cstf[:, 1] = w2.reshape(HID)
    return {"cstb": np.ascontiguousarray(
                cstb.astype(ml_dtypes.float8_e4m3)),
            "cstf": np.ascontiguousarray(cstf)}


def kernel(x, w1, b1, w2, b2):
    x = np.asarray(x, np.float32)
    assert x.shape == (B, C, L), x.shape
    nc = _get_compiled(np.float32(np.asarray(b2).reshape(-1)[0]))
    consts = _make_consts(w1, b1, w2, b2)
    xb = np.ascontiguousarray(x).astype(ml_dtypes.bfloat16)
    in_maps = []
    for i in range(N_CORES):
        m = {"x": np.ascontiguousarray(xb[i * BS:(i + 1) * BS])}
        m.update(consts)
        in_maps.append(m)
    res = run_bass_kernel_spmd(nc, in_maps, list(range(N_CORES)),
                               trace=bool(int(os.environ.get("K_TRACE", "0"))))
    out = np.concatenate(
        [np.asarray(res.results[i]["out"]).astype(np.float32)
         for i in range(N_CORES)], axis=0)
    if res.exec_time_ns is not None:
        kernel.last_exec_time_ns = res.exec_time_ns
        kernel.last_mean_exec_time_ns = res.mean_exec_time_ns
    kernel.last_results = res
    return out
